# revision 1
# baseline (speedup 1.0000x reference)
"""Trainium2 Bass kernel for nn_MultiHeadGATMethod (4-head GAT message passing).

Self-contained: host preprocessing (edge sort/shard/block-pack), bass kernel
build (SPMD over 8 NeuronCores), PJRT execution under axon, output assembly.

Algorithm: edges sorted by destination and sharded across 8 cores by disjoint
destination-node ranges (no collectives). Dense phase computes bf16 tables
K~=lrelu(x@Wk.T), H=x@Wh.T (all nodes, 4 heads packed, 1KB rows) and local
Q~=lrelu(x@Wq.T). Edge phase: dma_gather (int16 idx; 4 strided table views
handle >32K rows) pulls K~/H rows by source and Q~ rows by destination;
scores via DVE mul+reduce, softmax without max-subtraction (bounded scores,
divide by z at block drain), aggregation + per-head z via one PE matmul per
128-edge tile against a one-hot selection matrix, accumulated in PSUM per
destination block. Host adds mean bias (sum(alpha)=1 identity).
"""
import sys
sys.path.insert(0, "/opt/trn_rl_repo")
sys.path.insert(0, "/root/.axon_site/_ro/trn_rl_repo")

import sys
sys.path.insert(0, "/opt/trn_rl_repo")
import time
import numpy as np
import jax
from jax.sharding import Mesh, PartitionSpec, NamedSharding
from jax.experimental.shard_map import shard_map

import concourse.mybir as mybir
from concourse import bass2jax
from concourse.bass2jax import _bass_exec_p, partition_id_tensor, install_neuronx_cc_hook


class AxRunner:
    def __init__(self, nc, n_cores):
        install_neuronx_cc_hook()
        self.nc = nc
        self.n_cores = n_cores
        partition_name = nc.partition_id_tensor.name if nc.partition_id_tensor else None
        in_names, out_names, out_avals, zero_outs = [], [], [], []
        for alloc in nc.m.functions[0].allocations:
            if not isinstance(alloc, mybir.MemoryLocationSet):
                continue
            name = alloc.memorylocations[0].name
            if alloc.kind == "ExternalInput":
                if name != partition_name:
                    in_names.append(name)
            elif alloc.kind == "ExternalOutput":
                out_names.append(name)
                shape = tuple(alloc.tensor_shape)
                dtype = mybir.dt.np(alloc.dtype)
                out_avals.append(jax.core.ShapedArray(shape, dtype))
                zero_outs.append((shape, dtype))
        self.in_names, self.out_names = in_names, out_names
        self.out_avals = out_avals
        n_params = len(in_names)
        all_in_names = list(in_names) + list(out_names)
        if partition_name is not None:
            all_in_names.append(partition_name)

        def _body(*args):
            operands = list(args)
            if partition_name is not None:
                operands.append(partition_id_tensor())
            outs = _bass_exec_p.bind(
                *operands,
                out_avals=tuple(out_avals),
                in_names=tuple(all_in_names),
                out_names=tuple(out_names),
                lowering_input_output_aliases=(),
                sim_require_finite=False,
                sim_require_nnan=False,
                nc=nc,
            )
            return tuple(outs)

        devices = jax.devices()[:n_cores]
        self.mesh = Mesh(np.asarray(devices), ("core",))
        spec = PartitionSpec("core")
        in_specs = (spec,) * (n_params + len(out_names))
        out_specs = (spec,) * len(out_names)
        self.fn = jax.jit(
            shard_map(_body, mesh=self.mesh, in_specs=in_specs,
                      out_specs=out_specs, check_rep=False),
            keep_unused=True,
        )
        self.sharding = NamedSharding(self.mesh, spec)
        self._dev_in = None

    def put_inputs(self, in_maps):
        """in_maps: list (len n_cores) of {name: np.ndarray}."""
        concat = [
            np.concatenate([np.asarray(in_maps[c][nm]) for c in range(self.n_cores)], axis=0)
            for nm in self.in_names
        ]
        self._dev_in = [jax.device_put(a, self.sharding) for a in concat]
        self._dev_zeros = [
            jax.device_put(
                np.zeros((self.n_cores * s[0], *s[1:]), d), self.sharding
            )
            for (s, d) in [( (a[0]), a[1]) for a in [( (za[0]), za[1]) for za in []]]
        ]
        # build zero outputs (device-resident, re-usable because no donation)
        self._dev_zeros = []
        for aval_i, (shape, dtype) in enumerate(
            [(tuple(av.shape), av.dtype) for av in self.out_avals]
        ):
            z = np.zeros((self.n_cores * shape[0], *shape[1:]), dtype)
            self._dev_zeros.append(jax.device_put(z, self.sharding))
        jax.block_until_ready(self._dev_in)

    def run(self):
        outs = self.fn(*self._dev_in, *self._dev_zeros)
        jax.block_until_ready(outs)
        return outs

    def results(self, outs):
        res = []
        for c in range(self.n_cores):
            d = {}
            for i, nm in enumerate(self.out_names):
                a = np.asarray(outs[i])
                shape = tuple(self.out_avals[i].shape)
                d[nm] = a.reshape(self.n_cores, *shape)[c]
            res.append(d)
        return res

    def time_runs(self, iters=5):
        self.run()  # warmup/compile
        ts = []
        for _ in range(iters):
            t0 = time.perf_counter()
            self.run()
            ts.append(time.perf_counter() - t0)
        return min(ts), ts



import sys
sys.path.insert(0, "/opt/trn_rl_repo")
import numpy as np
import ml_dtypes

import concourse.bass as bass
import concourse.bacc as bacc
import concourse.mybir as mybir
import concourse.tile as tile

BF16 = mybir.dt.bfloat16
F32 = mybir.dt.float32
I16 = mybir.dt.int16

N_CORES = 8
N_HEADS = 4
OUT_DIM = 64
IN_DIM = 128
NEG_SLOPE = 0.2
NCHUNK = 4
TILE = 128
CALL = 1024          # gather idxs per dma_gather call
GHOST = 30000.0      # rrel value for pad slots (matches no node 0..127)


# ---------------------------------------------------------------- host side

def preprocess(x, edge_index, Wq, Wk, Wh, bh, tiles_per_chunk=3):
    """Returns (meta, per_core_inputs:list[dict])."""
    N = x.shape[0]
    E = edge_index.shape[1]
    row = np.asarray(edge_index[0], dtype=np.int64)
    col = np.asarray(edge_index[1], dtype=np.int64)
    perm = np.argsort(row, kind="stable")
    rs = row[perm].astype(np.int32)
    cs = col[perm].astype(np.int32)

    # core boundaries on destination nodes, ~equal edges
    nb = [0]
    for c in range(1, N_CORES):
        t = (E * c) // N_CORES
        node = int(rs[t])
        node = max(node, nb[-1] + 1)
        nb.append(min(node, N - 1))
    nb.append(N)
    e0 = [int(np.searchsorted(rs, nb[c], "left")) for c in range(N_CORES)] + [E]

    CAP = tiles_per_chunk * TILE  # per-(block, chunk) slot cap
    cores = []
    for c in range(N_CORES):
        lo, hi = e0[c], e0[c + 1]
        rl = rs[lo:hi] - nb[c]          # local dest ids, sorted
        cl = cs[lo:hi]
        nloc = nb[c + 1] - nb[c]
        # per (node, chunk) counts
        key = rl.astype(np.int64) * NCHUNK + (cl % NCHUNK)
        cnt = np.bincount(key, minlength=nloc * NCHUNK).reshape(nloc, NCHUNK)
        # greedy block packing
        blocks = []  # (start_node, n_nodes)
        s = 0
        while s < nloc:
            acc = np.zeros(NCHUNK, dtype=np.int64)
            n = 0
            while s + n < nloc and n < TILE:
                nxt = acc + cnt[s + n]
                if np.any(nxt > CAP):
                    break
                acc = nxt
                n += 1
            assert n > 0, "single node exceeds chunk cap"
            blocks.append((s, n))
            s += n
        cores.append(dict(nb=nb[c], nloc=nloc, rl=rl, cl=cl, cnt=cnt, blocks=blocks))

    B = max(len(ci["blocks"]) for ci in cores)
    NLOC = max(ci["nloc"] for ci in cores)
    NLOC = -(-NLOC // TILE) * TILE
    # chunk slot space: B blocks x CAP, padded to CALL multiple
    SC = B * CAP
    SCP = -(-SC // CALL) * CALL
    ncalls = SCP // CALL
    NPAD = -(-N // TILE) * TILE

    meta = dict(N=N, E=E, NPAD=NPAD, NLOC=NLOC, B=B, SC=SC, SCP=SCP,
                ncalls=ncalls, tiles_per_chunk=tiles_per_chunk, CAP=CAP,
                nb=nb, cores=cores)

    # dense-phase shared inputs
    xb = np.asarray(x, dtype=np.float32)
    xT = np.zeros((IN_DIM, NPAD), dtype=ml_dtypes.bfloat16)
    xT[:, :N] = xb.T.astype(ml_dtypes.bfloat16)
    Wk_ = np.asarray(Wk, np.float32)
    Wh_ = np.asarray(Wh, np.float32)
    Wq_ = np.asarray(Wq, np.float32)
    w_kh = np.concatenate(
        [Wk_[h].T for h in range(N_HEADS)] + [Wh_[h].T for h in range(N_HEADS)],
        axis=1).astype(ml_dtypes.bfloat16)          # [128, 512]
    w_q = np.concatenate([Wq_[h].T for h in range(N_HEADS)], axis=1).astype(
        ml_dtypes.bfloat16)                          # [128, 256]

    per_core = []
    for c in range(N_CORES):
        ci = cores[c]
        nloc, rl, cl, blocks = ci["nloc"], ci["rl"], ci["cl"], ci["blocks"]
        # order edges by (block, chunk): stable sort by chunk within node-sorted
        chunk = (cl % NCHUNK).astype(np.int64)
        blk_of_node = np.zeros(nloc, dtype=np.int64)
        for bi, (s, n) in enumerate(blocks):
            blk_of_node[s:s + n] = bi
        ekey = (blk_of_node[rl] * NCHUNK + chunk)
        eperm = np.argsort(ekey, kind="stable")
        rl2 = rl[eperm]
        cl2 = cl[eperm]
        ekey2 = ekey[eperm]
        # per (block, chunk) segment -> slots
        kh_idx = np.zeros((NCHUNK, SCP), dtype=np.int16)
        qx_idx = np.zeros((NCHUNK, SCP), dtype=np.int16)
        rrel = np.full((NCHUNK, SCP), GHOST, dtype=np.float32)
        bounds = np.searchsorted(ekey2, np.arange(len(blocks) * NCHUNK + 1))
        for bi in range(len(blocks)):
            s_node = blocks[bi][0]
            for k in range(NCHUNK):
                a, b_ = bounds[bi * NCHUNK + k], bounds[bi * NCHUNK + k + 1]
                n_e = b_ - a
                assert n_e <= CAP
                base = bi * CAP
                kh_idx[k, base:base + n_e] = (cl2[a:b_] // NCHUNK).astype(np.int16)
                qx_idx[k, base:base + n_e] = rl2[a:b_].astype(np.int16)
                rrel[k, base:base + n_e] = (rl2[a:b_] - s_node).astype(np.float32)
        # wrap idx arrays per call: [ncalls, 128, CALL//16] with slot i at
        # (i%16 of 16-partition group, i//16), replicated 8x down partitions
        def wrap(a):
            w = a.reshape(NCHUNK, ncalls, CALL // 16, 16).transpose(0, 1, 3, 2)
            return np.broadcast_to(
                w[:, :, None, :, :], (NCHUNK, ncalls, 8, 16, CALL // 16)
            ).reshape(NCHUNK, ncalls, TILE, CALL // 16).copy()
        kh_in = wrap(kh_idx)
        qx_in = wrap(qx_idx)
        # rrel in processing-tile layout: [128, ntiles] where tile
        # gtile=(b*NCHUNK+k)*tpc+t covers chunk-k slots (b*tpc+t)*128 + p
        tpc = tiles_per_chunk
        ntiles = B * NCHUNK * tpc
        rr = np.full((TILE, ntiles), GHOST, dtype=np.float32)
        for k in range(NCHUNK):
            v = rrel[k, :B * CAP].reshape(B, tpc, TILE)  # [b, t, p]
            for bi in range(B):
                for t in range(tpc):
                    rr[:, (bi * NCHUNK + k) * tpc + t] = v[bi, t]

        xTloc = np.zeros((IN_DIM, NLOC), dtype=ml_dtypes.bfloat16)
        sl = xb[nb[c]:nb[c + 1]].T.astype(ml_dtypes.bfloat16)
        xTloc[:, :nloc] = sl
        per_core.append(dict(xT=xT, xTloc=xTloc, w_kh=w_kh, w_q=w_q,
                             kh_idx=kh_in, qx_idx=qx_in, rrel=rr))
    return meta, per_core


def assemble(meta, results, bh):
    """results: list per core of {'res': [B*128, 64] bf16}. Returns [N, 64] f32."""
    N = meta["N"]
    out = np.zeros((N, OUT_DIM), dtype=np.float32)
    bias = np.asarray(bh, np.float32).mean(axis=0)
    deg = np.zeros(N, dtype=np.int64)
    for c in range(N_CORES):
        ci = meta["cores"][c]
        res = np.asarray(results[c]["res"], dtype=np.float32)
        for bi, (s, n) in enumerate(ci["blocks"]):
            out[meta["nb"][c] + s: meta["nb"][c] + s + n] = res[bi * TILE: bi * TILE + n]
        deg_l = np.bincount(ci["rl"], minlength=ci["nloc"])
        deg[meta["nb"][c]: meta["nb"][c] + ci["nloc"]] = deg_l
    out += bias[None, :]
    out[deg == 0] = 0.0
    return out


# -------------------------------------------------------------- device side

def build_kernel(meta, n_cores=N_CORES, nq=2):
    NPAD, NLOC, B = meta["NPAD"], meta["NLOC"], meta["B"]
    SCP, ncalls, tpc = meta["SCP"], meta["ncalls"], meta["tiles_per_chunk"]
    CAPk = tpc * TILE
    KHW = 2 * N_HEADS * OUT_DIM          # 512
    QW = N_HEADS * OUT_DIM               # 256
    RHSW = QW + N_HEADS                  # 260
    CPB = CALL // TILE                   # tiles per gather call
    KHW2 = KHW

    nc = bacc.Bacc("TRN2", target_bir_lowering=False, debug=False,
                   num_devices=n_cores, num_swdge_queues=nq)
    xT = nc.dram_tensor("xT", [IN_DIM, NPAD], BF16, kind="ExternalInput")
    xTloc = nc.dram_tensor("xTloc", [IN_DIM, NLOC], BF16, kind="ExternalInput")
    w_kh = nc.dram_tensor("w_kh", [IN_DIM, KHW], BF16, kind="ExternalInput")
    w_q = nc.dram_tensor("w_q", [IN_DIM, QW], BF16, kind="ExternalInput")
    kh_idx = nc.dram_tensor("kh_idx", [NCHUNK, ncalls, TILE, CALL // 16], I16,
                            kind="ExternalInput")
    qx_idx = nc.dram_tensor("qx_idx", [NCHUNK, ncalls, TILE, CALL // 16], I16,
                            kind="ExternalInput")
    rrel = nc.dram_tensor("rrel", [TILE, B * NCHUNK * tpc], F32,
                          kind="ExternalInput")
    res = nc.dram_tensor("res", [B * TILE, OUT_DIM], BF16, kind="ExternalOutput")
    KH = nc.dram_tensor("KH", [NPAD, KHW], BF16, kind="Internal")
    QT = nc.dram_tensor("QT", [NLOC, QW], BF16, kind="Internal")

    with tile.TileContext(nc) as tc:
        with (
            tc.tile_pool(name="dense", bufs=3) as dp,
            tc.tile_pool(name="psum", bufs=4, space="PSUM") as pp,
        ):
            # ---------------- dense phase ----------------
            wkh_t = dp.tile([IN_DIM, KHW], BF16, tag="wkh")
            nc.sync.dma_start(out=wkh_t[:], in_=w_kh[:, :])
            wq_t = dp.tile([IN_DIM, QW], BF16, tag="wq")
            nc.sync.dma_start(out=wq_t[:], in_=w_q[:, :])

            XB = 8  # node tiles per x-load batch
            nt = NPAD // TILE
            for j0 in range(0, nt, XB):
                jn = min(XB, nt - j0)
                xt = dp.tile([IN_DIM, XB * TILE], BF16, tag="xt")
                nc.sync.dma_start(out=xt[:, :jn * TILE],
                                  in_=xT[:, j0 * TILE:(j0 + jn) * TILE])
                st = dp.tile([TILE, XB, KHW], BF16, tag="khst")
                for j in range(jn):
                    ps = pp.tile([TILE, KHW], F32, tag="pdense")
                    nc.tensor.matmul(ps[:], lhsT=xt[:, j * TILE:(j + 1) * TILE],
                                     rhs=wkh_t[:], start=True, stop=True)
                    # K~ halves get lrelu (ACT), H halves copied (DVE)
                    nc.scalar.activation(st[:, j, 0:QW], ps[:, 0:QW],
                                         mybir.ActivationFunctionType.Prelu,
                                         alpha=NEG_SLOPE)
                    nc.vector.tensor_copy(st[:, j, QW:KHW], ps[:, QW:KHW])
                nc.sync.dma_start(
                    out=KH[j0 * TILE:(j0 + jn) * TILE, :].rearrange(
                        "(j p) d -> p j d", p=TILE),
                    in_=st[:, :jn, :])
            ntl = NLOC // TILE
            for j0 in range(0, ntl, XB):
                jn = min(XB, ntl - j0)
                xt = dp.tile([IN_DIM, XB * TILE], BF16, tag="xt")
                nc.sync.dma_start(out=xt[:, :jn * TILE],
                                  in_=xTloc[:, j0 * TILE:(j0 + jn) * TILE])
                st = dp.tile([TILE, XB, QW], BF16, tag="qst")
                for j in range(jn):
                    ps = pp.tile([TILE, QW], F32, tag="pq")
                    nc.tensor.matmul(ps[:], lhsT=xt[:, j * TILE:(j + 1) * TILE],
                                     rhs=wq_t[:], start=True, stop=True)
                    nc.scalar.activation(st[:, j, :], ps[:],
                                         mybir.ActivationFunctionType.Prelu,
                                         alpha=NEG_SLOPE)
                nc.sync.dma_start(
                    out=QT[j0 * TILE:(j0 + jn) * TILE, :].rearrange(
                        "(j p) d -> p j d", p=TILE),
                    in_=st[:, :jn, :])

        with (
            tc.tile_pool(name="gth", bufs=2) as gp,
            tc.tile_pool(name="aux", bufs=2) as ap,
            tc.tile_pool(name="ptile", bufs=8) as ptp,
            tc.tile_pool(name="small", bufs=1) as sp,
            tc.tile_pool(name="spsum", bufs=4, space="PSUM") as pp2,
            tc.tile_pool(name="outp", bufs=2) as op,
        ):
            # ---------------- edge phase ----------------
            iota_bf = sp.tile([TILE, TILE], F32)
            iota_i = sp.tile([TILE, TILE], mybir.dt.int32)
            nc.gpsimd.iota(iota_i[:], pattern=[[1, TILE]], base=0,
                           channel_multiplier=0)
            nc.vector.tensor_copy(iota_bf[:], iota_i[:])
            rrel_t = sp.tile([TILE, B * NCHUNK * tpc], F32)
            nc.sync.dma_start(out=rrel_t[:], in_=rrel[:, :])

            state = {}

            def issue_call(k, j):
                """Gather call j of chunk k + batch-level score ops."""
                it1t = ap.tile([TILE, CALL // 16], I16, tag=f"khidx{k}")
                nc.sync.dma_start(out=it1t[:], in_=kh_idx[k, j, :, :])
                it2t = ap.tile([TILE, CALL // 16], I16, tag=f"qxidx{k}")
                nc.sync.dma_start(out=it2t[:], in_=qx_idx[k, j, :, :])
                it1, it2 = it1t[:], it2t[:]
                khg = gp.tile([TILE, CPB, KHW], BF16, tag=f"khg{k}")
                nc.gpsimd.dma_gather(
                    out_ap=khg[:], in_ap=KH[k::NCHUNK, :], idxs_ap=it1,
                    num_idxs=CALL, num_idxs_reg=CALL, elem_size=KHW,
                    elem_step=NCHUNK * KHW, single_packet=False,
                    queue_num=(2 * j) % nq)
                qxg = gp.tile([TILE, CPB, QW], BF16, tag=f"qxg{k}")
                nc.gpsimd.dma_gather(
                    out_ap=qxg[:], in_ap=QT[:, :], idxs_ap=it2,
                    num_idxs=CALL, num_idxs_reg=CALL, elem_size=QW,
                    single_packet=False, queue_num=(2 * j + 1) % nq)
                qk = ap.tile([TILE, CPB, QW], BF16, tag=f"qk{k}")
                nc.vector.tensor_tensor(out=qk[:], in0=qxg[:],
                                        in1=khg[:, :, 0:QW],
                                        op=mybir.AluOpType.mult)
                s4 = ap.tile([TILE, CPB, N_HEADS], F32, tag=f"s4{k}")
                nc.vector.tensor_reduce(
                    out=s4[:],
                    in_=qk[:].rearrange("p c (h d) -> p c h d", d=OUT_DIM),
                    axis=mybir.AxisListType.X, op=mybir.AluOpType.add)
                es = ap.tile([TILE, CPB, N_HEADS], BF16, tag=f"es{k}")
                nc.scalar.activation(es[:], s4[:],
                                     mybir.ActivationFunctionType.Exp,
                                     scale=1.0 / (OUT_DIM ** 0.5))
                return khg, es

            def get_call(k, j):
                if (k, j) not in state:
                    state[(k, j)] = issue_call(k, j)
                return state[(k, j)]

            ostage = None
            OB = 8  # blocks per output stage
            for b in range(B):
                if b % OB == 0:
                    ostage = op.tile([TILE, OB, OUT_DIM], BF16, tag="ostage")
                psb = pp2.tile([TILE, RHSW], F32, tag="pblk")
                for k in range(NCHUNK):
                    for t in range(tpc):
                        gslot = (b * tpc + t) * TILE
                        j, colr = divmod(gslot, CALL)
                        colc = colr // TILE
                        khg, es = get_call(k, j)
                        gtile = (b * NCHUNK + k) * tpc + t
                        rhs = ptp.tile([TILE, RHSW], BF16, tag="rhst")
                        # es*H into rhs[:, 0:QW] in one broadcast TT
                        nc.vector.tensor_tensor(
                            out=rhs[:, 0:QW].rearrange("p (h d) -> p h d", d=OUT_DIM),
                            in0=khg[:, colc, QW:KHW2].rearrange(
                                "p (h d) -> p h d", d=OUT_DIM),
                            in1=es[:, colc, :].to_broadcast(
                                [TILE, N_HEADS, OUT_DIM]),
                            op=mybir.AluOpType.mult)
                        nc.scalar.copy(rhs[:, QW:RHSW], es[:, colc, :])
                        S = ptp.tile([TILE, TILE], BF16, tag="S")
                        nc.vector.tensor_scalar(
                            out=S[:], in0=iota_bf[:],
                            scalar1=rrel_t[:, gtile:gtile + 1], scalar2=None,
                            op0=mybir.AluOpType.is_equal)
                        nc.tensor.matmul(psb[:], lhsT=S[:], rhs=rhs[:],
                                         start=(k == 0 and t == 0),
                                         stop=(k == NCHUNK - 1 and t == tpc - 1))
                # drain block: out = 0.25 * sum_h psum[:, 64h:64h+64] / z_h
                zt = ap.tile([TILE, N_HEADS], F32, tag="zt")
                nc.vector.tensor_scalar(out=zt[:], in0=psb[:, QW:RHSW],
                                        scalar1=1e-20, scalar2=None,
                                        op0=mybir.AluOpType.add)
                rz = ap.tile([TILE, N_HEADS], F32, tag="rz")
                nc.vector.reciprocal(rz[:], zt[:])
                t4 = ap.tile([TILE, N_HEADS, OUT_DIM], F32, tag="t4")
                for h in range(N_HEADS):
                    nc.scalar.mul(t4[:, h, :], psb[:, h * OUT_DIM:(h + 1) * OUT_DIM],
                                  rz[:, h:h + 1])
                t2 = ap.tile([TILE, 2, OUT_DIM], F32, tag="t2")
                nc.vector.tensor_tensor(out=t2[:], in0=t4[:, 0:2, :],
                                        in1=t4[:, 2:4, :], op=mybir.AluOpType.add)
                t1 = ap.tile([TILE, OUT_DIM], F32, tag="t1")
                nc.vector.tensor_tensor(out=t1[:], in0=t2[:, 0, :], in1=t2[:, 1, :],
                                        op=mybir.AluOpType.add)
                nc.scalar.activation(ostage[:, b % OB, :], t1[:],
                                     mybir.ActivationFunctionType.Copy,
                                     scale=1.0 / N_HEADS)
                if b % OB == OB - 1 or b == B - 1:
                    b0 = (b // OB) * OB
                    bn = b - b0 + 1
                    nc.sync.dma_start(
                        out=res[b0 * TILE:(b0 + bn) * TILE, :].rearrange(
                            "(j p) d -> p j d", p=TILE),
                        in_=ostage[:, :bn, :])
    nc.compile()
    return nc


# ------------------------------------------------------------ entry point

_CACHE = {}


def _pick_tpc(edge_index, n_nodes, base=3):
    row = np.asarray(edge_index[0], dtype=np.int64)
    col = np.asarray(edge_index[1], dtype=np.int64)
    key = row * NCHUNK + (col % NCHUNK)
    mx = int(np.bincount(key, minlength=n_nodes * NCHUNK).max())
    return max(base, -(-mx // TILE))


def kernel(x, edge_index, Wq, Wk, Wh, bh):
    x = np.asarray(x)
    edge_index = np.asarray(edge_index)
    meta, per_core = preprocess(x, edge_index, Wq, Wk, Wh, bh,
                                tiles_per_chunk=_pick_tpc(edge_index, x.shape[0]))
    key = (meta["NPAD"], meta["NLOC"], meta["B"], meta["SCP"],
           meta["tiles_per_chunk"])
    if key not in _CACHE:
        nc = build_kernel(meta)
        _CACHE[key] = AxRunner(nc, N_CORES)
    r = _CACHE[key]
    r.put_inputs(per_core)
    results = r.results(r.run())
    return assemble(meta, results, bh).astype(np.float32)



# revision 2
# speedup vs baseline: 1.7393x; 1.7393x over previous
"""Trainium2 Bass kernel for nn_MultiHeadGATMethod (4-head GAT message passing).

Self-contained: host preprocessing (edge sort/shard/block-pack), bass kernel
build (SPMD over 8 NeuronCores), PJRT execution under axon, output assembly.

Algorithm: edges sorted by destination and sharded across 8 cores by disjoint
destination-node ranges (no collectives). Dense phase computes bf16 tables
K~=lrelu(x@Wk.T), H=x@Wh.T (4 per-residue tensors holding only the source
nodes this core's edges reference, dense-remapped int16 ids) and local
Q~=lrelu(x@Wq.T), with matched pairs of node tiles batched per ACT/DVE
instruction. Edge phase: dma_gather pulls K~/H rows by source and Q~ rows by
destination; scores via one 2x-rate bf16 DVE multiply + in-place pairwise
fold tree (64->32->16->8) + short f32 reduce; exp on ACT writes a 32-wide
replicated es so the es*H staging multiplies also run at 2x; softmax without
max-subtraction (bounded scores), per-head z and aggregation via one PE
matmul per 128-edge tile against a one-hot selection matrix (built at 4x
DVE rate from a bf16 iota), accumulated in PSUM per destination block and
divided by z at drain. Host adds mean bias (sum(alpha)=1 identity).
Engine notes: gpsimd is used ONLY for dma_gather/iota - real-HW gpsimd
tensor ops are far slower than the cost model suggests.
"""
import sys
sys.path.insert(0, "/opt/trn_rl_repo")
sys.path.insert(0, "/root/.axon_site/_ro/trn_rl_repo")

import sys
sys.path.insert(0, "/opt/trn_rl_repo")
import time
import numpy as np
import jax
from jax.sharding import Mesh, PartitionSpec, NamedSharding
from jax.experimental.shard_map import shard_map

import concourse.mybir as mybir
from concourse import bass2jax
from concourse.bass2jax import _bass_exec_p, partition_id_tensor, install_neuronx_cc_hook


class AxRunner:
    def __init__(self, nc, n_cores):
        install_neuronx_cc_hook()
        self.nc = nc
        self.n_cores = n_cores
        partition_name = nc.partition_id_tensor.name if nc.partition_id_tensor else None
        in_names, out_names, out_avals, zero_outs = [], [], [], []
        for alloc in nc.m.functions[0].allocations:
            if not isinstance(alloc, mybir.MemoryLocationSet):
                continue
            name = alloc.memorylocations[0].name
            if alloc.kind == "ExternalInput":
                if name != partition_name:
                    in_names.append(name)
            elif alloc.kind == "ExternalOutput":
                out_names.append(name)
                shape = tuple(alloc.tensor_shape)
                dtype = mybir.dt.np(alloc.dtype)
                out_avals.append(jax.core.ShapedArray(shape, dtype))
                zero_outs.append((shape, dtype))
        self.in_names, self.out_names = in_names, out_names
        self.out_avals = out_avals
        n_params = len(in_names)
        all_in_names = list(in_names) + list(out_names)
        if partition_name is not None:
            all_in_names.append(partition_name)

        def _body(*args):
            operands = list(args)
            if partition_name is not None:
                operands.append(partition_id_tensor())
            outs = _bass_exec_p.bind(
                *operands,
                out_avals=tuple(out_avals),
                in_names=tuple(all_in_names),
                out_names=tuple(out_names),
                lowering_input_output_aliases=(),
                sim_require_finite=False,
                sim_require_nnan=False,
                nc=nc,
            )
            return tuple(outs)

        devices = jax.devices()[:n_cores]
        self.mesh = Mesh(np.asarray(devices), ("core",))
        spec = PartitionSpec("core")
        in_specs = (spec,) * (n_params + len(out_names))
        out_specs = (spec,) * len(out_names)
        self.fn = jax.jit(
            shard_map(_body, mesh=self.mesh, in_specs=in_specs,
                      out_specs=out_specs, check_rep=False),
            keep_unused=True,
        )
        self.sharding = NamedSharding(self.mesh, spec)
        self._dev_in = None

    def put_inputs(self, in_maps):
        """in_maps: list (len n_cores) of {name: np.ndarray}."""
        concat = [
            np.concatenate([np.asarray(in_maps[c][nm]) for c in range(self.n_cores)], axis=0)
            for nm in self.in_names
        ]
        self._dev_in = [jax.device_put(a, self.sharding) for a in concat]
        self._dev_zeros = [
            jax.device_put(
                np.zeros((self.n_cores * s[0], *s[1:]), d), self.sharding
            )
            for (s, d) in [( (a[0]), a[1]) for a in [( (za[0]), za[1]) for za in []]]
        ]
        # build zero outputs (device-resident, re-usable because no donation)
        self._dev_zeros = []
        for aval_i, (shape, dtype) in enumerate(
            [(tuple(av.shape), av.dtype) for av in self.out_avals]
        ):
            z = np.zeros((self.n_cores * shape[0], *shape[1:]), dtype)
            self._dev_zeros.append(jax.device_put(z, self.sharding))
        jax.block_until_ready(self._dev_in)

    def run(self):
        outs = self.fn(*self._dev_in, *self._dev_zeros)
        jax.block_until_ready(outs)
        return outs

    def results(self, outs):
        res = []
        for c in range(self.n_cores):
            d = {}
            for i, nm in enumerate(self.out_names):
                a = np.asarray(outs[i])
                shape = tuple(self.out_avals[i].shape)
                d[nm] = a.reshape(self.n_cores, *shape)[c]
            res.append(d)
        return res

    def time_runs(self, iters=5):
        self.run()  # warmup/compile
        ts = []
        for _ in range(iters):
            t0 = time.perf_counter()
            self.run()
            ts.append(time.perf_counter() - t0)
        return min(ts), ts



import sys
sys.path.insert(0, "/opt/trn_rl_repo")
import numpy as np
import ml_dtypes

import concourse.bass as bass
import concourse.bacc as bacc
import concourse.mybir as mybir
import concourse.tile as tile

BF16 = mybir.dt.bfloat16
F32 = mybir.dt.float32
I16 = mybir.dt.int16

N_CORES = 8
N_HEADS = 4
OUT_DIM = 64
IN_DIM = 128
NEG_SLOPE = 0.2
NCHUNK = 4
TILE = 128
CALL = 1024          # gather idxs per dma_gather call
GHOST = 30000.0      # rrel value for pad slots (matches no node 0..127)


# ---------------------------------------------------------------- host side

def preprocess(x, edge_index, Wq, Wk, Wh, bh, tiles_per_chunk=3):
    """Returns (meta, per_core_inputs:list[dict])."""
    N = x.shape[0]
    E = edge_index.shape[1]
    row = np.asarray(edge_index[0], dtype=np.int64)
    col = np.asarray(edge_index[1], dtype=np.int64)
    perm = np.argsort(row, kind="stable")
    rs = row[perm].astype(np.int32)
    cs = col[perm].astype(np.int32)

    # core boundaries on destination nodes, ~equal edges
    nb = [0]
    for c in range(1, N_CORES):
        t = (E * c) // N_CORES
        node = int(rs[t])
        node = max(node, nb[-1] + 1)
        nb.append(min(node, N - 1))
    nb.append(N)
    e0 = [int(np.searchsorted(rs, nb[c], "left")) for c in range(N_CORES)] + [E]

    CAP = tiles_per_chunk * TILE  # per-(block, chunk) slot cap
    cores = []
    for c in range(N_CORES):
        lo, hi = e0[c], e0[c + 1]
        rl = rs[lo:hi] - nb[c]          # local dest ids, sorted
        cl = cs[lo:hi]
        nloc = nb[c + 1] - nb[c]
        # per (node, chunk) counts
        key = rl.astype(np.int64) * NCHUNK + (cl % NCHUNK)
        cnt = np.bincount(key, minlength=nloc * NCHUNK).reshape(nloc, NCHUNK)
        # greedy block packing
        blocks = []  # (start_node, n_nodes)
        s = 0
        while s < nloc:
            acc = np.zeros(NCHUNK, dtype=np.int64)
            n = 0
            while s + n < nloc and n < TILE:
                nxt = acc + cnt[s + n]
                if np.any(nxt > CAP):
                    break
                acc = nxt
                n += 1
            assert n > 0, "single node exceeds chunk cap"
            blocks.append((s, n))
            s += n
        cores.append(dict(nb=nb[c], nloc=nloc, rl=rl, cl=cl, cnt=cnt, blocks=blocks))

    B = max(len(ci["blocks"]) for ci in cores)
    NLOC = max(ci["nloc"] for ci in cores)
    NLOC = -(-NLOC // TILE) * TILE
    # chunk slot space: B blocks x CAP, padded to CALL multiple
    SC = B * CAP
    SCP = -(-SC // CALL) * CALL
    ncalls = SCP // CALL
    NPAD = -(-N // TILE) * TILE

    # dense-phase inputs: per-core, per-residue tables holding ONLY the
    # source nodes that core's edges reference (~86% of N), remapped to
    # dense ids; K/H is computed per chunk as one contiguous tensor so
    # chunk-k gathers can start while later chunks are still being computed
    xb = np.asarray(x, dtype=np.float32)
    xbh = xb.astype(ml_dtypes.bfloat16)
    core_ids = []  # [core][k] -> referenced residue-row ids (sorted)
    for c in range(N_CORES):
        cl = cores[c]["cl"]
        ids = [np.unique(cl[cl % NCHUNK == k] // NCHUNK)
               for k in range(NCHUNK)]
        core_ids.append(ids)
    NPADR = max(-(-len(ids_k) // TILE) * TILE
                for ids in core_ids for ids_k in ids)

    meta = dict(N=N, E=E, NPAD=NPAD, NPADR=NPADR, NLOC=NLOC, B=B, SC=SC,
                SCP=SCP, ncalls=ncalls, tiles_per_chunk=tiles_per_chunk,
                CAP=CAP, nb=nb, cores=cores)

    Wk_ = np.asarray(Wk, np.float32)
    Wh_ = np.asarray(Wh, np.float32)
    Wq_ = np.asarray(Wq, np.float32)
    w_kh = np.concatenate(
        [Wk_[h].T for h in range(N_HEADS)] + [Wh_[h].T for h in range(N_HEADS)],
        axis=1).astype(ml_dtypes.bfloat16)          # [128, 512]
    w_q = np.concatenate([Wq_[h].T for h in range(N_HEADS)], axis=1).astype(
        ml_dtypes.bfloat16)                          # [128, 256]

    per_core = []
    for c in range(N_CORES):
        ci = cores[c]
        nloc, rl, cl, blocks = ci["nloc"], ci["rl"], ci["cl"], ci["blocks"]
        # residue-row remap to dense ids + per-core x table
        NRES = -(-N // NCHUNK)
        remap = np.zeros((NCHUNK, NRES), dtype=np.int16)
        xTc = np.zeros((IN_DIM, NCHUNK, NPADR), dtype=ml_dtypes.bfloat16)
        for k in range(NCHUNK):
            ids_k = core_ids[c][k]
            remap[k, ids_k] = np.arange(len(ids_k), dtype=np.int16)
            orig = ids_k * NCHUNK + k
            orig = orig[orig < N]
            xTc[:, k, :len(orig)] = xbh[orig].T
        # order edges by (block, chunk): stable sort by chunk within node-sorted
        chunk = (cl % NCHUNK).astype(np.int64)
        blk_of_node = np.zeros(nloc, dtype=np.int64)
        for bi, (s, n) in enumerate(blocks):
            blk_of_node[s:s + n] = bi
        ekey = (blk_of_node[rl] * NCHUNK + chunk)
        eperm = np.argsort(ekey, kind="stable")
        rl2 = rl[eperm]
        cl2 = cl[eperm]
        ekey2 = ekey[eperm]
        # per (block, chunk) segment -> slots
        kh_idx = np.zeros((NCHUNK, SCP), dtype=np.int16)
        qx_idx = np.zeros((NCHUNK, SCP), dtype=np.int16)
        rrel = np.full((NCHUNK, SCP), GHOST, dtype=np.float32)  # cast bf16 below
        bounds = np.searchsorted(ekey2, np.arange(len(blocks) * NCHUNK + 1))
        for bi in range(len(blocks)):
            s_node = blocks[bi][0]
            for k in range(NCHUNK):
                a, b_ = bounds[bi * NCHUNK + k], bounds[bi * NCHUNK + k + 1]
                n_e = b_ - a
                assert n_e <= CAP
                base = bi * CAP
                kh_idx[k, base:base + n_e] = remap[k, cl2[a:b_] // NCHUNK]
                qx_idx[k, base:base + n_e] = rl2[a:b_].astype(np.int16)
                rrel[k, base:base + n_e] = (rl2[a:b_] - s_node).astype(np.float32)
        # wrap idx arrays per call: [ncalls, 128, CALL//16] with slot i at
        # (i%16 of 16-partition group, i//16), replicated 8x down partitions
        def wrap(a):
            w = a.reshape(NCHUNK, ncalls, CALL // 16, 16).transpose(0, 1, 3, 2)
            return np.broadcast_to(
                w[:, :, None, :, :], (NCHUNK, ncalls, 8, 16, CALL // 16)
            ).reshape(NCHUNK, ncalls, TILE, CALL // 16).copy()
        kh_in = wrap(kh_idx)
        qx_in = wrap(qx_idx)
        # rrel in processing-tile layout: [128, ntiles] where tile
        # gtile=(b*NCHUNK+k)*tpc+t covers chunk-k slots (b*tpc+t)*128 + p
        tpc = tiles_per_chunk
        ntiles = B * NCHUNK * tpc
        rr = np.full((TILE, ntiles), GHOST, dtype=np.float32)
        for k in range(NCHUNK):
            v = rrel[k, :B * CAP].reshape(B, tpc, TILE)  # [b, t, p]
            for bi in range(B):
                for t in range(tpc):
                    rr[:, (bi * NCHUNK + k) * tpc + t] = v[bi, t]

        xTloc = np.zeros((IN_DIM, NLOC), dtype=ml_dtypes.bfloat16)
        sl = xb[nb[c]:nb[c + 1]].T.astype(ml_dtypes.bfloat16)
        xTloc[:, :nloc] = sl
        per_core.append(dict(xT=xTc, xTloc=xTloc, w_kh=w_kh, w_q=w_q,
                             kh_idx=kh_in, qx_idx=qx_in, rrel=rr))
    return meta, per_core


def assemble(meta, results, bh):
    """results: list per core of {'res': [B*128, 64] bf16}. Returns [N, 64] f32."""
    N = meta["N"]
    out = np.zeros((N, OUT_DIM), dtype=np.float32)
    bias = np.asarray(bh, np.float32).mean(axis=0)
    deg = np.zeros(N, dtype=np.int64)
    for c in range(N_CORES):
        ci = meta["cores"][c]
        res = np.asarray(results[c]["res"], dtype=np.float32)
        for bi, (s, n) in enumerate(ci["blocks"]):
            out[meta["nb"][c] + s: meta["nb"][c] + s + n] = res[bi * TILE: bi * TILE + n]
        deg_l = np.bincount(ci["rl"], minlength=ci["nloc"])
        deg[meta["nb"][c]: meta["nb"][c] + ci["nloc"]] = deg_l
    out += bias[None, :]
    out[deg == 0] = 0.0
    return out


# -------------------------------------------------------------- device side

def build_kernel(meta, n_cores=N_CORES, nq=2):
    NPAD, NLOC, B = meta["NPAD"], meta["NLOC"], meta["B"]
    NPADR = meta["NPADR"]
    SCP, ncalls, tpc = meta["SCP"], meta["ncalls"], meta["tiles_per_chunk"]
    CAPk = tpc * TILE
    KHW = 2 * N_HEADS * OUT_DIM          # 512
    QW = N_HEADS * OUT_DIM               # 256
    RHSW = QW + N_HEADS                  # 260
    CPB = CALL // TILE                   # tiles per gather call
    KHW2 = KHW

    nc = bacc.Bacc("TRN2", target_bir_lowering=False, debug=False,
                   num_devices=n_cores, num_swdge_queues=nq)
    xT = nc.dram_tensor("xT", [IN_DIM, NCHUNK, NPADR], BF16,
                        kind="ExternalInput")
    xTloc = nc.dram_tensor("xTloc", [IN_DIM, NLOC], BF16, kind="ExternalInput")
    w_kh = nc.dram_tensor("w_kh", [IN_DIM, KHW], BF16, kind="ExternalInput")
    w_q = nc.dram_tensor("w_q", [IN_DIM, QW], BF16, kind="ExternalInput")
    kh_idx = nc.dram_tensor("kh_idx", [NCHUNK, ncalls, TILE, CALL // 16], I16,
                            kind="ExternalInput")
    qx_idx = nc.dram_tensor("qx_idx", [NCHUNK, ncalls, TILE, CALL // 16], I16,
                            kind="ExternalInput")
    rrel = nc.dram_tensor("rrel", [TILE, B * NCHUNK * tpc], F32,
                          kind="ExternalInput")
    res = nc.dram_tensor("res", [B * TILE, OUT_DIM], BF16, kind="ExternalOutput")
    KHk = [nc.dram_tensor(f"KH{k}", [NPADR, KHW], BF16, kind="Internal")
           for k in range(NCHUNK)]
    QT = nc.dram_tensor("QT", [NLOC, QW], BF16, kind="Internal")

    with tile.TileContext(nc) as tc:
        with (
            tc.tile_pool(name="dense", bufs=3) as dp,
            tc.tile_pool(name="psumd", bufs=2, space="PSUM") as ppd,
            tc.tile_pool(name="psumq", bufs=2, space="PSUM") as ppq,
        ):
            # ---------------- dense phase ----------------
            wkh_t = dp.tile([IN_DIM, KHW], BF16, tag="wkh")
            nc.sync.dma_start(out=wkh_t[:], in_=w_kh[:, :])
            wq_t = dp.tile([IN_DIM, QW], BF16, tag="wq")
            nc.sync.dma_start(out=wq_t[:], in_=w_q[:, :])

            XB = 8  # node tiles per x-load batch
            ntl = NLOC // TILE
            for j0 in range(0, ntl, XB):
                jn = min(XB, ntl - j0)
                xt = dp.tile([IN_DIM, XB * TILE], BF16, tag="xt")
                nc.sync.dma_start(out=xt[:, :jn * TILE],
                                  in_=xTloc[:, j0 * TILE:(j0 + jn) * TILE])
                st = dp.tile([TILE, XB, QW], BF16, tag="qst")
                for jp in range(0, jn, 2):
                    pn = min(2, jn - jp)
                    ps = ppq.tile([TILE, 2, QW], F32, tag="pq")
                    for j in range(pn):
                        nc.tensor.matmul(
                            ps[:, j, :],
                            lhsT=xt[:, (jp + j) * TILE:(jp + j + 1) * TILE],
                            rhs=wq_t[:], start=True, stop=True)
                    nc.scalar.activation(st[:, jp:jp + pn, :], ps[:, 0:pn, :],
                                         mybir.ActivationFunctionType.Prelu,
                                         alpha=NEG_SLOPE)
                nc.sync.dma_start(
                    out=QT[j0 * TILE:(j0 + jn) * TILE, :].rearrange(
                        "(j p) d -> p j d", p=TILE),
                    in_=st[:, :jn, :])

            nt = NPADR // TILE
            for kk in range(NCHUNK):
                for j0 in range(0, nt, XB):
                    jn = min(XB, nt - j0)
                    xt = dp.tile([IN_DIM, XB * TILE], BF16, tag="xt")
                    nc.sync.dma_start(out=xt[:, :jn * TILE],
                                      in_=xT[:, kk, j0 * TILE:(j0 + jn) * TILE])
                    st = dp.tile([TILE, XB, KHW], BF16, tag="khst")
                    for jp in range(0, jn, 2):
                        pn = min(2, jn - jp)
                        ps = ppd.tile([TILE, 2, KHW], F32, tag="pdense")
                        for j in range(pn):
                            nc.tensor.matmul(
                                ps[:, j, :],
                                lhsT=xt[:, (jp + j) * TILE:(jp + j + 1) * TILE],
                                rhs=wkh_t[:], start=True, stop=True)
                        # K~ halves get lrelu (ACT), H halves copied (DVE)
                        nc.scalar.activation(st[:, jp:jp + pn, 0:QW],
                                             ps[:, 0:pn, 0:QW],
                                             mybir.ActivationFunctionType.Prelu,
                                             alpha=NEG_SLOPE)
                        nc.vector.tensor_copy(st[:, jp:jp + pn, QW:KHW],
                                              ps[:, 0:pn, QW:KHW])
                    nc.sync.dma_start(
                        out=KHk[kk][j0 * TILE:(j0 + jn) * TILE, :].rearrange(
                            "(j p) d -> p j d", p=TILE),
                        in_=st[:, :jn, :])
        with (
            tc.tile_pool(name="gth", bufs=2) as gp,
            tc.tile_pool(name="aux", bufs=2) as ap,
            tc.tile_pool(name="ptile", bufs=8) as ptp,
            tc.tile_pool(name="small", bufs=1) as sp,
            tc.tile_pool(name="spsum", bufs=4, space="PSUM") as pp2,
            tc.tile_pool(name="outp", bufs=2) as op,
        ):
            # ---------------- edge phase ----------------
            iota_bf = sp.tile([TILE, TILE], BF16)
            iota_i = sp.tile([TILE, TILE], mybir.dt.int32)
            nc.gpsimd.iota(iota_i[:], pattern=[[1, TILE]], base=0,
                           channel_multiplier=0)
            nc.vector.tensor_copy(iota_bf[:], iota_i[:])
            rrel_t = sp.tile([TILE, B * NCHUNK * tpc], F32)
            nc.sync.dma_start(out=rrel_t[:], in_=rrel[:, :])

            state = {}
            FD = 16  # post-fold depth per head (64 -> 32 -> 16)

            def score_call(k, j):
                """Gather call j of chunk k + batch-level score ops."""
                it1t = ap.tile([TILE, CALL // 16], I16, tag=f"khidx{k}")
                nc.sync.dma_start(out=it1t[:], in_=kh_idx[k, j, :, :])
                it2t = ap.tile([TILE, CALL // 16], I16, tag=f"qxidx{k}")
                nc.sync.dma_start(out=it2t[:], in_=qx_idx[k, j, :, :])
                it1, it2 = it1t[:], it2t[:]
                khg = gp.tile([TILE, CPB, KHW], BF16, tag=f"khg{k}")
                nc.gpsimd.dma_gather(
                    out_ap=khg[:], in_ap=KHk[k][:, :], idxs_ap=it1,
                    num_idxs=CALL, num_idxs_reg=CALL, elem_size=KHW,
                    single_packet=False,
                    queue_num=(2 * j) % nq)
                qxg = gp.tile([TILE, CPB, QW], BF16, tag=f"qxg{k}")
                nc.gpsimd.dma_gather(
                    out_ap=qxg[:], in_ap=QT[:, :], idxs_ap=it2,
                    num_idxs=CALL, num_idxs_reg=CALL, elem_size=QW,
                    single_packet=False, queue_num=(2 * j + 1) % nq)
                qk = ap.tile([TILE, CPB, QW], BF16, tag=f"qk{k}")
                nc.vector.tensor_tensor(out=qk[:], in0=qxg[:],
                                        in1=khg[:, :, 0:QW],
                                        op=mybir.AluOpType.mult)
                # in-place pairwise fold tree 64 -> 32 -> 16 -> 8 at 2x DVE
                # rate, then an 8-wide f32 reduce: much cheaper than a
                # 64-wide 1x reduce
                qk4 = qk[:].rearrange("p c (h d) -> p c h d", d=OUT_DIM)
                nc.vector.tensor_tensor(out=qk4[:, :, :, 0:32],
                                        in0=qk4[:, :, :, 0:32],
                                        in1=qk4[:, :, :, 32:64],
                                        op=mybir.AluOpType.add)
                nc.vector.tensor_tensor(out=qk4[:, :, :, 0:16],
                                        in0=qk4[:, :, :, 0:16],
                                        in1=qk4[:, :, :, 16:32],
                                        op=mybir.AluOpType.add)
                nc.vector.tensor_tensor(out=qk4[:, :, :, 0:8],
                                        in0=qk4[:, :, :, 0:8],
                                        in1=qk4[:, :, :, 8:16],
                                        op=mybir.AluOpType.add)
                s4 = ap.tile([TILE, CPB, N_HEADS], F32, tag=f"s4{k}")
                nc.vector.tensor_reduce(
                    out=s4[:], in_=qk4[:, :, :, 0:8],
                    axis=mybir.AxisListType.X, op=mybir.AluOpType.add)
                # exp with 32-wide replication (stride-0 input) so the es*H
                # multiply sees a packed bf16 operand and runs at 2x
                es32 = ap.tile([TILE, CPB, N_HEADS, 32], BF16, tag=f"es{k}")
                nc.scalar.activation(
                    es32[:], s4[:].to_broadcast([TILE, CPB, N_HEADS, 32]),
                    mybir.ActivationFunctionType.Exp,
                    scale=1.0 / (OUT_DIM ** 0.5))
                return {"khg": khg, "es32": es32, "rhs8": None}

            def rhs_call(k, ent):
                # rhs staging for all CPB tiles of this call: es*H | es
                khg, es32 = ent["khg"], ent["es32"]
                rhs8 = ap.tile([TILE, CPB, RHSW], BF16, tag=f"rhs{k}")
                rhsv = rhs8[:][:, :, 0:QW].rearrange(
                    "p c (h d) -> p c h d", d=OUT_DIM)
                khv = khg[:, :, QW:KHW2].rearrange(
                    "p c (h d) -> p c h d", d=OUT_DIM)
                nc.vector.tensor_tensor(out=rhsv[:, :, :, 0:32],
                                        in0=khv[:, :, :, 0:32],
                                        in1=es32[:], op=mybir.AluOpType.mult)
                nc.vector.tensor_tensor(out=rhsv[:, :, :, 32:64],
                                        in0=khv[:, :, :, 32:64],
                                        in1=es32[:], op=mybir.AluOpType.mult)
                nc.scalar.copy(rhs8[:][:, :, QW:RHSW], es32[:][:, :, :, 0])
                return rhs8

            def get_call(k, j):
                if (k, j) not in state:
                    state[(k, j)] = score_call(k, j)
                # pipeline the next call's score stage so the DVE queue has
                # work while this call's exp runs on ACT
                if j + 1 < ncalls and (k, j + 1) not in state:
                    state[(k, j + 1)] = score_call(k, j + 1)
                ent = state[(k, j)]
                if ent["rhs8"] is None:
                    ent["rhs8"] = rhs_call(k, ent)
                return ent["rhs8"]

            ostage = None
            OB = 4  # blocks per output stage
            for b in range(B):
                if b % OB == 0:
                    ostage = op.tile([TILE, OB, OUT_DIM], BF16, tag="ostage")
                psb = pp2.tile([TILE, RHSW], F32, tag="pblk")
                for k in range(NCHUNK):
                    for t in range(tpc):
                        gslot = (b * tpc + t) * TILE
                        j, colr = divmod(gslot, CALL)
                        colc = colr // TILE
                        rhs8 = get_call(k, j)
                        gtile = (b * NCHUNK + k) * tpc + t
                        S = ptp.tile([TILE, TILE], BF16, tag="S")
                        nc.vector.tensor_scalar(
                            out=S[:], in0=iota_bf[:],
                            scalar1=rrel_t[:, gtile:gtile + 1], scalar2=None,
                            op0=mybir.AluOpType.is_equal)
                        nc.tensor.matmul(psb[:], lhsT=S[:],
                                         rhs=rhs8[:][:, colc, :],
                                         start=(k == 0 and t == 0),
                                         stop=(k == NCHUNK - 1 and t == tpc - 1))
                # drain block: out = 0.25 * sum_h psum[:, 64h:64h+64] / z_h
                rz = ap.tile([TILE, N_HEADS], F32, tag="rz")
                nc.vector.reciprocal(rz[:], psb[:, QW:RHSW])
                t4 = ap.tile([TILE, N_HEADS, OUT_DIM], F32, tag="t4")
                for h in range(N_HEADS):
                    nc.scalar.mul(t4[:, h, :], psb[:, h * OUT_DIM:(h + 1) * OUT_DIM],
                                  rz[:, h:h + 1])
                t2 = ap.tile([TILE, 2, OUT_DIM], F32, tag="t2")
                nc.vector.tensor_tensor(out=t2[:], in0=t4[:, 0:2, :],
                                        in1=t4[:, 2:4, :], op=mybir.AluOpType.add)
                t1 = ap.tile([TILE, OUT_DIM], F32, tag="t1")
                nc.vector.tensor_tensor(out=t1[:], in0=t2[:, 0, :], in1=t2[:, 1, :],
                                        op=mybir.AluOpType.add)
                nc.scalar.activation(ostage[:, b % OB, :], t1[:],
                                     mybir.ActivationFunctionType.Copy,
                                     scale=1.0 / N_HEADS)
                if b % OB == OB - 1 or b == B - 1:
                    b0 = (b // OB) * OB
                    bn = b - b0 + 1
                    nc.sync.dma_start(
                        out=res[b0 * TILE:(b0 + bn) * TILE, :].rearrange(
                            "(j p) d -> p j d", p=TILE),
                        in_=ostage[:, :bn, :])
    nc.compile()
    return nc


# ------------------------------------------------------------ entry point

_CACHE = {}


def _pick_tpc(edge_index, n_nodes, base=3):
    row = np.asarray(edge_index[0], dtype=np.int64)
    col = np.asarray(edge_index[1], dtype=np.int64)
    key = row * NCHUNK + (col % NCHUNK)
    mx = int(np.bincount(key, minlength=n_nodes * NCHUNK).max())
    return max(base, -(-mx // TILE))


def kernel(x, edge_index, Wq, Wk, Wh, bh):
    x = np.asarray(x)
    edge_index = np.asarray(edge_index)
    meta, per_core = preprocess(x, edge_index, Wq, Wk, Wh, bh,
                                tiles_per_chunk=_pick_tpc(edge_index, x.shape[0]))
    key = (meta["NPAD"], meta["NLOC"], meta["B"], meta["SCP"],
           meta["tiles_per_chunk"])
    if key not in _CACHE:
        nc = build_kernel(meta)
        _CACHE[key] = AxRunner(nc, N_CORES)
    r = _CACHE[key]
    r.put_inputs(per_core)
    results = r.results(r.run())
    return assemble(meta, results, bh).astype(np.float32)

